# revision 1
# baseline (speedup 1.0000x reference)
"""Trainium2 Bass kernel for the BalSCL/SSL balanced supervised-contrastive loss.

Distribution: data-parallel over the 8192 anchor rows, 1024 rows per core on
8 NeuronCores.  Each core computes a partial loss numerator / denominator and
the host combines the 8 scalar pairs.

Math (restructured from the reference, analytically identical):
  N = 8292 columns (8192 anchors + 100 class centers), all unit-norm.
  The row-max subtraction in the reference cancels analytically, so
    loss_i = log(S_i) - (10/m_i) * Sm_i
  with
    S_i  = sum_{j != i} exp(10 * f_i . g_j) / (cc_j - [lab_j == lab_i])
    Sm_i = sum_{j != i, lab_j == lab_i} f_i . g_j
    m_i  = cc[lab_i] - 1      (number of positive pairs for row i)
  Using the one-hot structure everything reduces to per-class aggregates on
  the tensor engine:
    E[c, i]   = sum_{j in class c} exp(10 * rawT[j, i])     (incl. j == i)
    gsum[c,:] = sum_{j in class c} g_j ;  gath[:, i] = gsum[lab_i, :]
  and the diagonal (j == i) contribution is subtracted analytically using
  ||f_i||^2, re-quantized to bf16 so it matches the bf16-stored exp that
  entered E bit-for-bit.  Per-row gathers over classes are one-hot matmuls;
  1/m comes from a per-class constant vector (no reciprocal needed).  The
  final numerator is sum(conf*ln(S)) - sum(conf*SmT); conf (a 0/1 mask) is
  folded into S' = conf*S + (1-conf) so the Ln activation's accumulator
  yields sum(conf*ln(S)) directly.
"""

import os
import sys

sys.path.insert(0, "/opt/trn_rl_repo")

import numpy as np
import ml_dtypes

import concourse.bass as bass  # noqa: F401
import concourse.bacc as bacc
import concourse.tile as tile
from concourse import mybir
from concourse.bass_utils import run_bass_kernel_spmd

F32 = mybir.dt.float32
BF16 = mybir.dt.bfloat16
BF = ml_dtypes.bfloat16
AF = mybir.ActivationFunctionType
ALU = mybir.AluOpType

B2, C, D = 8192, 100, 128
TEMP = 0.1
N = B2 + C                # 8292
TJ = (N + 127) // 128     # 65 j-tiles
NPAD = TJ * 128           # 8320
CORES = 8
R = B2 // CORES           # 1024 rows per core
CH = 512                  # i-chunk width (one fp32 PSUM bank)
GW = 3                    # j-tiles per exp group (3 PSUM banks)
GROUPS = [(g * GW, min(GW, TJ - g * GW)) for g in range((TJ + GW - 1) // GW)]
N_WARM = 7                # PE warm-up matmuls (HAM un-throttle)

FLAG_LNACC = os.environ.get("KB_LNACC", "1") == "1"
FLAG_ONETAB = os.environ.get("KB_ONETAB", "1") == "1"

_NC_CACHE = {}

# Prefer the combined exp+ln activation-table set so the kernel needs a single
# ACT_TABLE_LOAD instead of an exp-set load plus a mid-stream ln-set reload.
_orig_gat = bacc.get_activation_tables


def _gat_combined(arch):
    tabs = _orig_gat(arch)
    if not FLAG_ONETAB:
        return tabs
    out = {}
    for name, funcs in tabs.items():
        if name in ("exp_and_others", "exp_and_friends", "natural_log"):
            out[name] = set()  # keep position (set ids are positional)
        else:
            out[name] = funcs
    return out


def _build_nc():
    bacc.get_activation_tables = _gat_combined
    try:
        return _build_nc_inner()
    finally:
        bacc.get_activation_tables = _orig_gat


def _build_nc_inner():
    nc = bacc.Bacc()

    fTg = nc.dram_tensor("fTg", [D, NPAD], BF16, kind="ExternalInput")
    fAn = nc.dram_tensor("fAn", [128, TJ * 128], BF16, kind="ExternalInput")
    TAg = nc.dram_tensor("TAg", [128, TJ * C], BF16, kind="ExternalInput")
    fTc = nc.dram_tensor("fTc", [D, R], BF16, kind="ExternalInput")
    tTp = nc.dram_tensor("tTp", [C, R], BF16, kind="ExternalInput")
    W2 = nc.dram_tensor("W2", [C, R], F32, kind="ExternalInput")
    conf = nc.dram_tensor("conf", [1, R], F32, kind="ExternalInput")
    rcc = nc.dram_tensor("rcc", [C, 1], BF16, kind="ExternalInput")
    outd = nc.dram_tensor("out", [1, 2], F32, kind="ExternalOutput")

    with tile.TileContext(nc) as tc:
        with (
            tc.tile_pool(name="consts", bufs=1) as cp,
            tc.tile_pool(name="expp", bufs=6) as ep,
            tc.tile_pool(name="asmp", bufs=2) as am,
            tc.tile_pool(name="rawp", bufs=2, space="PSUM") as rp,
            tc.tile_pool(name="epsp", bufs=1, space="PSUM") as pp,
            tc.tile_pool(name="smp", bufs=1, space="PSUM") as sp,
        ):
            # ------------- input loads (ordered by first hardware use) ------
            s_fTc = cp.tile([D, R], BF16)
            s_fTg = cp.tile([D, NPAD], BF16)
            s_TAg = cp.tile([128, TJ * C], BF16)
            s_fAn = cp.tile([128, TJ * 128], BF16)
            nc.sync.dma_start(out=s_fTc[:, 0:CH], in_=fTc[:, 0:CH])
            nc.sync.dma_start(out=s_fTg[:, 0:1024], in_=fTg[:, 0:1024])
            nc.sync.dma_start(out=s_fTg[:, 1024:2560], in_=fTg[:, 1024:2560])
            nc.sync.dma_start(out=s_fTc[:, CH:R], in_=fTc[:, CH:R])
            s_tTp = cp.tile([C, R], BF16)
            nc.sync.dma_start(out=s_tTp, in_=tTp[:])
            s_rcc = cp.tile([C, 1], BF16)
            nc.sync.dma_start(out=s_rcc, in_=rcc[:])
            nc.sync.dma_start(out=s_TAg[:, 0 : 8 * C], in_=TAg[:, 0 : 8 * C])
            nc.sync.dma_start(out=s_fAn[:, 0:1024], in_=fAn[:, 0:1024])
            nc.sync.dma_start(out=s_fTg[:, 2560 : 36 * 128], in_=fTg[:, 2560 : 36 * 128])
            nc.sync.dma_start(out=s_TAg[:, 8 * C : 36 * C], in_=TAg[:, 8 * C : 36 * C])
            nc.sync.dma_start(out=s_fAn[:, 1024 : 36 * 128], in_=fAn[:, 1024 : 36 * 128])
            nc.sync.dma_start(out=s_fTg[:, 36 * 128 :], in_=fTg[:, 36 * 128 :])
            nc.sync.dma_start(out=s_TAg[:, 36 * C :], in_=TAg[:, 36 * C :])
            nc.sync.dma_start(out=s_fAn[:, 36 * 128 :], in_=fAn[:, 36 * 128 :])
            s_conf = cp.tile([1, R], F32)
            nc.sync.dma_start(out=s_conf, in_=conf[:])
            s_W2 = cp.tile([C, R], F32)
            nc.sync.dma_start(out=s_W2, in_=W2[:])

            s_ones = cp.tile([128, 1], F32)
            nc.vector.memset(s_ones, 1.0)
            s_ones_bf = cp.tile([128, 1], BF16)
            nc.vector.memset(s_ones_bf, 1.0)
            s_nones_bf = cp.tile([128, 1], BF16)
            nc.vector.memset(s_nones_bf, -1.0)

            s_gsum = cp.tile([C, D], BF16)
            s_scr = cp.tile([128, CH], BF16)
            nc.vector.memset(s_scr, 1.0)

            # PE warm-up in the DMA-wait window: HAM un-throttles ~3.4us in
            warmPS = sp.tile([128, CH], F32, name="warmPS", tag="sm")
            for _ in range(8):
                nc.tensor.matmul(
                    warmPS, lhsT=s_scr[:, 0:128], rhs=s_scr, start=True, stop=True
                )

            # conf denominator (off the critical tail)
            denv = am.tile([1, 1], F32)
            nc.vector.reduce_sum(out=denv, in_=s_conf, axis=mybir.AxisListType.X)

            # ------------- EPS-independent smalls (run in the DMA window) ----
            # minv10[i] = 10/(cc[lab_i]-1): exact per-class select
            minv = am.tile([1, R], F32, name="minv", tag="minv")
            for k in (0, 1):
                i0 = k * CH
                mPS = sp.tile([1, CH], F32, name=f"mPS{k}", tag="sm")
                nc.tensor.matmul(
                    mPS, lhsT=s_rcc, rhs=s_tTp[:, i0 : i0 + CH],
                    start=True, stop=True,
                )
                nc.vector.tensor_copy(minv[:, i0 : i0 + CH], mPS)

            # sq (f32, for fsq) + sq_bf (bf16, for the smr colsum)
            sq_bf = am.tile([128, R], BF16, name="sq_bf", tag="sq_bf")
            nc.vector.tensor_mul(sq_bf, s_fTc, s_fTc)
            dg_t = [None, None]
            for k in (0, 1):
                i0 = k * CH
                sq = am.tile([128, CH], F32, name=f"sq{k}", tag="sq")
                nc.vector.tensor_mul(
                    sq, s_fTc[:, i0 : i0 + CH], s_fTc[:, i0 : i0 + CH]
                )
                fsqPS = sp.tile([1, CH], F32, name=f"fsqPS{k}", tag="sm")
                nc.tensor.matmul(fsqPS, lhsT=s_ones, rhs=sq, start=True, stop=True)
                ed_bf = am.tile([1, CH], BF16, name=f"edb{k}", tag="edb")
                nc.scalar.activation(
                    out=ed_bf, in_=fsqPS, func=AF.Exp, scale=1.0 / TEMP
                )
                # dg = exp(10 fsq)/m ; with conf folding:
                #   e1 = (dg + 1)*conf - 1  so that  S' = conf*S + (1-conf)
                dg = am.tile([1, CH], F32, name=f"dg{k}", tag="dg")
                nc.vector.scalar_tensor_tensor(
                    out=dg, in0=ed_bf, scalar=0.1, in1=minv[:, i0 : i0 + CH],
                    op0=ALU.mult, op1=ALU.mult,
                )
                if FLAG_LNACC:
                    e1a = am.tile([1, CH], F32, name=f"e1a{k}", tag="e1a")
                    nc.vector.scalar_tensor_tensor(
                        out=e1a, in0=dg, scalar=1.0, in1=s_conf[:, i0 : i0 + CH],
                        op0=ALU.add, op1=ALU.mult,
                    )
                    e1 = am.tile([1, CH], F32, name=f"e1{k}", tag="e1")
                    nc.vector.tensor_scalar_add(e1, e1a, -1.0)
                    dg_t[k] = e1
                else:
                    dg_t[k] = dg

            # ------------- per-chunk raw/exp/E pipeline -------------
            def chunk_body(k, extras=()):
                i0 = k * CH
                extras = dict(extras)
                EPS = pp.tile([C, CH], F32, name=f"EPS{k}", tag="EPS")
                for gi, (t0, gw) in enumerate(GROUPS):
                    rawPS = rp.tile([128, CH * GW], F32, name="rawPS", tag="raw")
                    for q in range(gw):
                        t = t0 + q
                        nc.tensor.matmul(
                            rawPS[:, CH * q : CH * (q + 1)],
                            lhsT=s_fTg[:, 128 * t : 128 * (t + 1)],
                            rhs=s_fTc[:, i0 : i0 + CH],
                            start=True,
                            stop=True,
                        )
                    exps = ep.tile([128, CH * GW], BF16, name="exps", tag="exps")
                    nc.scalar.activation(
                        out=exps[:, : CH * gw],
                        in_=rawPS[:, : CH * gw],
                        func=AF.Exp,
                        scale=1.0 / TEMP,
                    )
                    for q in range(gw):
                        t = t0 + q
                        nc.tensor.matmul(
                            EPS,
                            lhsT=s_TAg[:, C * t : C * (t + 1)],
                            rhs=exps[:, CH * q : CH * (q + 1)],
                            start=(t == 0),
                            stop=(t == TJ - 1),
                        )
                    for fn in extras.pop(gi, ()):
                        fn()
                for fns in extras.values():
                    for fn in fns:
                        fn()
                return EPS

            # ---------------- gsum (interleaved with the chunks) -------------
            gsumPS = sp.tile([C, D], F32, name="gsumPS", tag="sm")
            gsum_state = {"t": 0}

            def gsum_step(n=2):
                def go():
                    t0 = gsum_state["t"]
                    for t in range(t0, min(t0 + n, TJ)):
                        nc.tensor.matmul(
                            gsumPS,
                            lhsT=s_TAg[:, C * t : C * (t + 1)],
                            rhs=s_fAn[:, 128 * t : 128 * (t + 1)],
                            start=(t == 0),
                            stop=(t == TJ - 1),
                        )
                    gsum_state["t"] = min(t0 + n, TJ)
                return go

            Sall = cp.tile([1, R], F32)

            def mk_srow(k, W2E):
                def go():
                    i0 = k * CH
                    SrowPS = sp.tile([1, CH], F32, name=f"SrowPS{k}", tag="sm")
                    nc.tensor.matmul(
                        SrowPS, lhsT=s_ones_bf[0:C, :], rhs=W2E, start=True, stop=True
                    )
                    if FLAG_LNACC:
                        Scm = am.tile([1, CH], F32, name=f"Scm{k}", tag="Scm")
                        nc.vector.tensor_mul(Scm, SrowPS, s_conf[:, i0 : i0 + CH])
                        nc.vector.tensor_sub(Sall[:, i0 : i0 + CH], Scm, dg_t[k])
                    else:
                        nc.vector.tensor_sub(
                            Sall[:, i0 : i0 + CH], SrowPS, dg_t[k]
                        )
                return go

            # Sm path: one N=1024 gather + fused (Asel - fsq) colsum, then
            # SmT = smr * minv and numB = sum(SmT * conf)
            numB = am.tile([1, 1], F32, name="numB", tag="numB")
            gm_state = {}

            s_SmT = cp.tile([1, R], F32)

            def mk_gath(k):
                def go():
                    i0 = k * CH
                    gathT = sp.tile([D, CH], F32, name=f"gathT{k}", tag="sm")
                    nc.tensor.matmul(
                        gathT, lhsT=s_gsum, rhs=s_tTp[:, i0 : i0 + CH],
                        start=True, stop=True,
                    )
                    gmul = am.tile([128, CH], BF16, name=f"gmul{k}", tag="gmul")
                    nc.vector.tensor_mul(gmul, gathT, s_fTc[:, i0 : i0 + CH])
                    gm_state[k] = gmul
                return go

            def mk_smr(k):
                def go():
                    i0 = k * CH
                    smrPS = sp.tile([1, CH], F32, name=f"smrPS{k}", tag="sm")
                    nc.tensor.matmul(
                        smrPS, lhsT=s_ones_bf, rhs=gm_state[k],
                        start=True, stop=False,
                    )
                    nc.tensor.matmul(
                        smrPS, lhsT=s_nones_bf, rhs=sq_bf[:, i0 : i0 + CH],
                        start=False, stop=True,
                    )
                    nc.vector.tensor_mul(
                        s_SmT[:, i0 : i0 + CH], smrPS, minv[:, i0 : i0 + CH]
                    )
                return go

            def mk_smtc():
                def go():
                    smtc = am.tile([1, R], F32, name="smtc", tag="smtc")
                    nc.vector.tensor_mul(smtc, s_SmT, s_conf)
                    nc.vector.reduce_sum(
                        out=numB, in_=smtc, axis=mybir.AxisListType.X
                    )
                return go

            # chunk 0: 2 gsum matmuls interleaved per group
            extras0 = {gi: [gsum_step(2)] for gi in range(1, 22)}
            EPS0 = chunk_body(0, extras=extras0)

            W2E0 = am.tile([C, CH], BF16, name="W2E0", tag="W2E")
            nc.vector.tensor_mul(W2E0, EPS0, s_W2[:, 0:CH])

            # chunk 1: finish gsum early, then the gsum-dependent smalls
            extras1 = {gi: [gsum_step(2)] for gi in range(1, 11)}
            extras1[11] = [gsum_step(TJ)]
            extras1.setdefault(12, []).append(
                lambda: nc.vector.tensor_copy(s_gsum, gsumPS)
            )
            extras1.setdefault(2, []).append(mk_srow(0, W2E0))
            extras1.setdefault(12, []).append(mk_gath(0))
            extras1.setdefault(13, []).append(mk_gath(1))
            extras1.setdefault(14, []).append(mk_smr(0))
            extras1.setdefault(15, []).append(mk_smr(1))
            extras1.setdefault(16, []).append(mk_smtc())
            EPS1 = chunk_body(1, extras=extras1)

            # ---------------- tail ----------------
            W2E1 = am.tile([C, CH], BF16, name="W2E1", tag="W2E")
            nc.vector.tensor_mul(W2E1, EPS1, s_W2[:, CH : 2 * CH])
            mk_srow(1, W2E1)()

            lg = am.tile([1, R], F32)
            numA = am.tile([1, 1], F32)
            if FLAG_LNACC:
                nc.scalar.activation(out=lg, in_=Sall, func=AF.Ln, accum_out=numA)
            else:
                nc.scalar.activation(out=lg, in_=Sall, func=AF.Ln)
                wrow = am.tile([1, R], F32)
                nc.vector.tensor_mul(wrow, lg, s_conf)
                nc.vector.reduce_sum(out=numA, in_=wrow, axis=mybir.AxisListType.X)
            numv = am.tile([1, 1], F32)
            nc.vector.tensor_sub(numv, numA, numB)
            outsb = am.tile([1, 2], F32)
            nc.vector.tensor_copy(outsb[:, 0:1], numv)
            nc.vector.tensor_copy(outsb[:, 1:2], denv)
            nc.sync.dma_start(out=outd[:], in_=outsb)

    nc.finalize()
    return nc


def _get_nc():
    if "nc" not in _NC_CACHE:
        _NC_CACHE["nc"] = _build_nc()
    return _NC_CACHE["nc"]


def _prep_inputs(centers1, features, targets, conf_mask):
    f32 = np.float32
    features = np.ascontiguousarray(features, dtype=f32)
    centers1 = np.ascontiguousarray(centers1, dtype=f32).reshape(-1, D)
    targets = np.ascontiguousarray(targets, dtype=f32)
    conf_mask = np.ascontiguousarray(conf_mask, dtype=f32)

    feats_all = np.concatenate([features, centers1], axis=0)  # [N, D]
    fa_pad = np.zeros((NPAD, D), dtype=f32)
    fa_pad[:N] = feats_all
    TA = np.concatenate([targets, np.eye(C, dtype=f32)], axis=0)  # [N, C]
    TA_pad = np.zeros((NPAD, C), dtype=f32)
    TA_pad[:N] = TA

    fTg_np = np.ascontiguousarray(fa_pad.T).astype(BF)  # [D, NPAD]
    fAn_np = np.ascontiguousarray(
        fa_pad.reshape(TJ, 128, D).transpose(1, 0, 2).reshape(128, TJ * D)
    ).astype(BF)
    TAg_np = np.ascontiguousarray(
        TA_pad.reshape(TJ, 128, C).transpose(1, 0, 2).reshape(128, TJ * C)
    ).astype(BF)

    cc = targets.sum(axis=0, dtype=np.float64) + 1.0  # [C]
    safe = cc > 1.5
    dcls = np.where(safe, 1.0 / np.maximum(cc - 1.0, 1.0) - 1.0 / cc, 0.0)
    invc = 1.0 / cc
    rcc_np = np.where(safe, 10.0 / np.maximum(cc - 1.0, 1.0), 0.0)
    rcc_np = rcc_np.astype(BF).reshape(C, 1)

    in_maps = []
    for c in range(CORES):
        rows = slice(c * R, (c + 1) * R)
        fTc_np = np.ascontiguousarray(fTg_np[:, c * R : (c + 1) * R])
        tTp_f32 = np.ascontiguousarray(targets[rows].T, dtype=f32)  # [C, R]
        tTp_np = tTp_f32.astype(BF)
        W2_np = (dcls[:, None] * tTp_f32 + invc[:, None]).astype(f32)
        conf_np = np.ascontiguousarray(conf_mask[rows].reshape(1, R), dtype=f32)
        in_maps.append(
            {
                "fTg": fTg_np,
                "fAn": fAn_np,
                "TAg": TAg_np,
                "fTc": fTc_np,
                "tTp": tTp_np,
                "W2": W2_np,
                "conf": conf_np,
                "rcc": rcc_np,
            }
        )
    return in_maps


def _run(centers1, features, targets, conf_mask, trace=False, trace_cores=None):
    in_maps = _prep_inputs(centers1, features, targets, conf_mask)
    nc = _get_nc()
    kwargs = {}
    if trace:
        # NTFF profiling under axon: shim the (absent) antenv.axon_hooks
        # module and skip the artifact bucket upload.
        import types
        import concourse.bass_utils as bass_utils

        if "antenv.axon_hooks" not in sys.modules:
            mod = types.ModuleType("antenv.axon_hooks")
            mod._hook = None

            def set_axon_ntff_profile_hook(h):
                mod._hook = h

            def get_axon_ntff_profile_hook():
                return mod._hook

            mod.set_axon_ntff_profile_hook = set_axon_ntff_profile_hook
            mod.get_axon_ntff_profile_hook = get_axon_ntff_profile_hook
            sys.modules["antenv.axon_hooks"] = mod
            from trn_agent_boot.trn_boot import _ntff_profile_via_ctypes

            set_axon_ntff_profile_hook(
                _ntff_profile_via_ctypes("/opt/axon/libaxon_pjrt.so")
            )
        bass_utils.upload_artifacts = lambda tmpdir: "local://" + tmpdir
        kwargs = {"trace": True}
        if trace_cores is not None:
            kwargs["trace_cores"] = trace_cores
    res = run_bass_kernel_spmd(nc, in_maps, core_ids=list(range(CORES)), **kwargs)
    num = 0.0
    den = 0.0
    for r in res.results:
        num += float(r["out"][0, 0])
        den += float(r["out"][0, 1])
    loss = np.array(num / den, dtype=np.float32)
    return loss, res


def kernel(centers1, features, targets, cls_num_list, conf_mask):
    loss, _ = _run(centers1, features, targets, conf_mask)
    return loss



# revision 10
# speedup vs baseline: 1.0451x; 1.0451x over previous
"""Trainium2 Bass kernel for the BalSCL/SSL balanced supervised-contrastive loss.

Distribution: data-parallel over the 8192 anchor rows, 1024 rows per core on
8 NeuronCores.

The device computes only the O(B2*N) part that needs hardware:
    S_i = sum_c W2[c,i] * EPS[c,i],  EPS[c,i] = sum_{j in c, j!=i} exp(10 f_i.g_j - SH)
and ships the per-row denominator S (one [1,1024] f32 row per core).  The
O(N*D) side terms (per-class feature sums, positive-pair numerator, conf
masking, the final log/mean) are exact closed-form dot products the host
computes in fp64 from the original inputs.

Device pipeline per core (2 i-chunks of 512 x 66 j-tiles):
  - raw logits: bf16 matmuls, [128, 3, 512] PSUM groups (3 j-tiles).
  - self-pair masking: each core's own 8 j-tiles are permuted to positions
    0..7 of its private input copies, so one SPMD program masks the
    diagonal stripes at fixed coordinates (raw -= 30 => exp == 0).
  - exp: split between ScalarE (HW exp -> fp8e4, bias -SH) and VectorE
    (one-op Schraudolph: uint8 bits = round(A*logit + B), bit-cast fp8e4;
    ~3% rms/elem, averages out in the ~83-term class sums).
  - EPS: fp8 DoubleRow matmuls, two j-tiles per instruction (the exp shift
    SH keeps every exp below the TRN fp8e4 Inf threshold of 240).
"""

import os
import sys

sys.path.insert(0, "/opt/trn_rl_repo")

import numpy as np
import ml_dtypes

import concourse.bass as bass  # noqa: F401
import concourse.bacc as bacc
import concourse.tile as tile
from concourse import mybir
from concourse.bass_utils import run_bass_kernel_spmd

F32 = mybir.dt.float32
BF16 = mybir.dt.bfloat16
F8 = mybir.dt.float8e4
U8 = mybir.dt.uint8
BF = ml_dtypes.bfloat16
F8NP = ml_dtypes.float8_e4m3
AF = mybir.ActivationFunctionType
ALU = mybir.AluOpType
DR = mybir.MatmulPerfMode.DoubleRow

B2, C, D = 8192, 100, 128
TEMP = 0.1
N = B2 + C                 # 8292
TJ = 66                    # j-tiles (padded: 8448 = 66*128)
NPAD = TJ * 128
Q = TJ // 2                # 33 j-tile pairs
G = TJ // 3                # 22 exp groups of 3 j-tiles
CORES = 8
R = B2 // CORES            # 1024 rows per core
CH = 512                   # i-chunk width (one fp32 PSUM bank)
CP = 112                   # class dim padded to a 16-byte multiple (fp8 DR lhsT)

SH = 1.25                  # global exp downscale: exp(logit - SH), host adds back
SCH_A = 8.0 / float(np.log(2.0)) * (1.0 / TEMP)   # applied to raw dot
SCH_B = 56.0 - 0.46 - (8.0 / float(np.log(2.0))) * SH

# Groups whose exp runs on VectorE (the rest on ScalarE).  Diagonal groups
# (chunk0: 0,1; chunk1: 1,2) must stay on ScalarE.
N_DVE_G = int(os.environ.get("KB_NDVEG", "10"))
# chunk -> group -> [(slot, 128-col block)] of the diagonal stripes
DIAG = {0: {0: [(0, 0), (1, 1), (2, 2)], 1: [(0, 3)]},
        1: {1: [(1, 0), (2, 1)], 2: [(0, 2), (1, 3)]}}


def _dve_groups(chunk, n):
    forced_act = set(DIAG[chunk].keys())
    elig = [g for g in range(G) if g not in forced_act]
    step = len(elig) / max(n, 1)
    return {elig[min(int((i + 0.5) * step), len(elig) - 1)] for i in range(n)}


_NC_CACHE = {}

# Combined exp+ln activation-table set: a single ACT_TABLE_LOAD.
_orig_gat = bacc.get_activation_tables


def _gat_combined(arch):
    tabs = _orig_gat(arch)
    out = {}
    for name, funcs in tabs.items():
        if name in ("exp_and_others", "exp_and_friends", "natural_log"):
            out[name] = set()  # keep position (set ids are positional)
        else:
            out[name] = funcs
    return out


def _build_nc():
    bacc.get_activation_tables = _gat_combined
    try:
        return _build_nc_inner()
    finally:
        bacc.get_activation_tables = _orig_gat


def _build_nc_inner():
    nc = bacc.Bacc()

    fTcd = nc.dram_tensor("fTc", [D, R], BF16, kind="ExternalInput")
    fTgd = nc.dram_tensor("fTg", [D, NPAD], BF16, kind="ExternalInput")
    dmaskd = nc.dram_tensor("dmask", [128, 128], BF16, kind="ExternalInput")
    TAgd = nc.dram_tensor("TAg8", [128, TJ, CP], F8, kind="ExternalInput")
    W2d = nc.dram_tensor("W2", [C, R], BF16, kind="ExternalInput")
    srowd = nc.dram_tensor("srow", [1, R], F32, kind="ExternalOutput")

    dve_sets = [_dve_groups(0, N_DVE_G), _dve_groups(1, N_DVE_G - 1)]

    with tile.TileContext(nc) as tc:
        with (
            tc.tile_pool(name="consts", bufs=1) as cp,
            tc.tile_pool(name="rawp", bufs=2, space="PSUM") as rp,
            tc.tile_pool(name="epsp", bufs=1, space="PSUM") as pp,
            tc.tile_pool(name="smp", bufs=1, space="PSUM") as sp,
        ):
            # ---------------- input DMA (ordered by first use) ----------
            s_fTc = cp.tile([D, R], BF16)
            nc.sync.dma_start(out=s_fTc, in_=fTcd[:])
            s_fTg = cp.tile([D, NPAD], BF16)
            s_TAg = cp.tile([128, TJ, CP], F8)
            nc.sync.dma_start(out=s_fTg[:, 0:1536], in_=fTgd[:, 0:1536])
            s_dmask = cp.tile([128, 128], BF16)
            nc.sync.dma_start(out=s_dmask, in_=dmaskd[:])
            nc.sync.dma_start(out=s_TAg[:, 0:12, :], in_=TAgd[:, 0:12, :])
            nc.sync.dma_start(out=s_fTg[:, 1536:4608], in_=fTgd[:, 1536:4608])
            nc.sync.dma_start(out=s_TAg[:, 12:36, :], in_=TAgd[:, 12:36, :])
            nc.sync.dma_start(out=s_fTg[:, 4608:NPAD], in_=fTgd[:, 4608:NPAD])
            nc.sync.dma_start(out=s_TAg[:, 36:TJ, :], in_=TAgd[:, 36:TJ, :])
            s_W2 = cp.tile([C, R], BF16)
            nc.sync.dma_start(out=s_W2, in_=W2d[:])

            s_negsh = cp.tile([128, 1], F32)
            nc.gpsimd.memset(s_negsh, -SH)
            s_ones_bf = cp.tile([128, 1], BF16)
            nc.gpsimd.memset(s_ones_bf, 1.0)

            # trigger the ACT table load in the DMA window
            tabt = cp.tile([1, 16], F32)
            nc.scalar.activation(
                out=tabt, in_=s_negsh[0:1, :].to_broadcast((1, 16)), func=AF.Exp
            )

            s_ex = [
                cp.tile([128, TJ, CH], F8, name="ex0"),
                cp.tile([128, TJ, CH], F8, name="ex1"),
            ]
            s_srow = cp.tile([1, R], F32)
            W2E = [None, None]

            # ---------------- per-chunk pipeline ------------------------
            def chunk_body(c, extras):
                i0 = c * CH
                dve_set = dve_sets[c]
                EPS = pp.tile([CP, CH], F32, name=f"EPS{c}", tag="EPS")
                q_done = 0
                for g in range(G):
                    rawT = rp.tile([128, 3, CH], F32, name="rawT", tag="raw")
                    for s in (0, 1, 2):
                        t = 3 * g + s
                        nc.tensor.matmul(
                            rawT[:, s, :],
                            lhsT=s_fTg[:, 128 * t : 128 * (t + 1)],
                            rhs=s_fTc[:, i0 : i0 + CH],
                            start=True, stop=True,
                        )
                    for s, b in DIAG[c].get(g, ()):
                        nc.vector.scalar_tensor_tensor(
                            out=rawT[:, s, 128 * b : 128 * (b + 1)],
                            in0=s_dmask, scalar=-30.0,
                            in1=rawT[:, s, 128 * b : 128 * (b + 1)],
                            op0=ALU.mult, op1=ALU.add,
                        )
                    exd = s_ex[c][:, 3 * g : 3 * g + 3, :]
                    if g in dve_set:
                        nc.vector.tensor_scalar(
                            exd.bitcast(U8), rawT[:, :, :],
                            SCH_A, SCH_B, op0=ALU.mult, op1=ALU.add,
                        )
                    else:
                        nc.scalar.activation(
                            out=exd, in_=rawT[:, :, :], func=AF.Exp,
                            scale=1.0 / TEMP, bias=s_negsh,
                        )
                    # EPS pairs fully covered by exp'd tiles so far
                    q_ready = min((3 * g + 3) // 2, Q)
                    for q in range(q_done, q_ready):
                        nc.tensor.matmul(
                            EPS,
                            lhsT=s_TAg[:, 2 * q : 2 * q + 2, :],
                            rhs=s_ex[c][:, 2 * q : 2 * q + 2, :],
                            start=(q == 0), stop=(q == Q - 1), perf_mode=DR,
                        )
                    q_done = q_ready
                    for fn in extras.pop(g, ()):
                        fn()
                for fns in extras.values():
                    for fn in fns:
                        fn()
                return EPS

            def mk_w2e(c, EPS):
                def go():
                    i0 = c * CH
                    w = cp.tile([C, CH], BF16, name=f"W2E{c}", tag=f"W2E{c}")
                    nc.vector.tensor_mul(w, EPS[0:C, :], s_W2[:, i0 : i0 + CH])
                    W2E[c] = w
                return go

            def mk_srow(c):
                def go():
                    i0 = c * CH
                    SrowPS = sp.tile([1, CH], F32, name=f"SrowPS{c}", tag="sm")
                    nc.tensor.matmul(
                        SrowPS, lhsT=s_ones_bf[0:C, :], rhs=W2E[c],
                        start=True, stop=True,
                    )
                    nc.vector.tensor_copy(s_srow[:, i0 : i0 + CH], SrowPS)
                return go

            EPS0 = chunk_body(0, {})
            mk_w2e(0, EPS0)()
            EPS1 = chunk_body(1, {2: [mk_srow(0)]})

            # ---------------- tail ----------------
            mk_w2e(1, EPS1)()
            mk_srow(1)()
            nc.sync.dma_start(out=srowd[:], in_=s_srow)

    nc.finalize()
    return nc


def _get_nc():
    if "nc" not in _NC_CACHE:
        _NC_CACHE["nc"] = _build_nc()
    return _NC_CACHE["nc"]


def _prep_inputs(centers1, features, targets):
    f32 = np.float32
    feats_all = np.concatenate([features, centers1], axis=0)  # [N, D]
    fa_pad = np.zeros((NPAD, D), dtype=f32)
    fa_pad[:N] = feats_all
    TA_pad = np.zeros((NPAD, C), dtype=f32)
    TA_pad[:B2] = targets
    TA_pad[B2:N] = np.eye(C, dtype=f32)

    fa_t = fa_pad.reshape(TJ, 128, D)
    TA_t = TA_pad.reshape(TJ, 128, C)

    cc = targets.sum(axis=0, dtype=np.float64) + 1.0  # [C]
    safe = cc > 1.5
    dcls = np.where(safe, 1.0 / np.maximum(cc - 1.0, 1.0) - 1.0 / cc, 0.0)
    invc = 1.0 / cc

    dmask_np = np.eye(128, dtype=f32).astype(BF)

    in_maps = []
    for k in range(CORES):
        rows = slice(k * R, (k + 1) * R)
        # permute j-tiles: this core's own 8 tiles go to positions 0..7
        own = list(range(8 * k, 8 * k + 8))
        perm = own + [t for t in range(TJ) if t not in own]
        fa_k = fa_t[perm]                      # [TJ, 128, D]
        TA_k = TA_t[perm]                      # [TJ, 128, C]

        fTg_np = np.ascontiguousarray(
            fa_k.transpose(2, 0, 1).reshape(D, NPAD)
        ).astype(BF)
        TA_kp = np.zeros((TJ, 128, CP), dtype=f32)
        TA_kp[:, :, :C] = TA_k
        TAg8_np = np.ascontiguousarray(TA_kp.transpose(1, 0, 2)).astype(F8NP)
        fTc_np = np.ascontiguousarray(features[rows].T).astype(BF)
        tTp_f32 = np.ascontiguousarray(targets[rows].T, dtype=f32)  # [C, R]
        W2_np = (dcls[:, None] * tTp_f32 + invc[:, None]).astype(BF)
        in_maps.append(
            {
                "fTc": fTc_np,
                "fTg": fTg_np,
                "dmask": dmask_np,
                "TAg8": TAg8_np,
                "W2": W2_np,
            }
        )
    return in_maps


def _run(centers1, features, targets, conf_mask, trace=False, trace_cores=None):
    f32 = np.float32
    features = np.ascontiguousarray(features, dtype=f32)
    centers1 = np.ascontiguousarray(centers1, dtype=f32).reshape(-1, D)
    targets = np.ascontiguousarray(targets, dtype=f32)
    conf = np.ascontiguousarray(conf_mask, dtype=np.float64)

    in_maps = _prep_inputs(centers1, features, targets)
    nc = _get_nc()
    kwargs = {}
    if trace:
        # NTFF profiling under axon: shim the (absent) antenv.axon_hooks
        # module and skip the artifact bucket upload.
        import types
        import concourse.bass_utils as bass_utils

        if "antenv.axon_hooks" not in sys.modules:
            mod = types.ModuleType("antenv.axon_hooks")
            mod._hook = None

            def set_axon_ntff_profile_hook(h):
                mod._hook = h

            def get_axon_ntff_profile_hook():
                return mod._hook

            mod.set_axon_ntff_profile_hook = set_axon_ntff_profile_hook
            mod.get_axon_ntff_profile_hook = get_axon_ntff_profile_hook
            sys.modules["antenv.axon_hooks"] = mod
            from trn_agent_boot.trn_boot import _ntff_profile_via_ctypes

            set_axon_ntff_profile_hook(
                _ntff_profile_via_ctypes("/opt/axon/libaxon_pjrt.so")
            )
        bass_utils.upload_artifacts = lambda tmpdir: "local://" + tmpdir
        kwargs = {"trace": True}
        if trace_cores is not None:
            kwargs["trace_cores"] = trace_cores
    res = run_bass_kernel_spmd(nc, in_maps, core_ids=list(range(CORES)), **kwargs)

    # host-side closed-form pieces (fp64, exact)
    S = np.concatenate([r["srow"][0] for r in res.results]).astype(np.float64)
    labs = targets.argmax(axis=1)
    cc = targets.sum(axis=0, dtype=np.float64) + 1.0
    gsum = targets.T.astype(np.float64) @ features.astype(np.float64)
    gsum += centers1.astype(np.float64)                      # [C, D]
    gath = gsum[labs]                                        # [B2, D]
    f64 = features.astype(np.float64)
    sm_raw = (f64 * gath).sum(axis=1) - (f64 * f64).sum(axis=1)
    SmT = (1.0 / TEMP) * sm_raw / (cc[labs] - 1.0)
    lnS = np.log(np.maximum(S, 1e-30)) + SH
    num = (conf * (lnS - SmT)).sum()
    den = conf.sum()
    loss = np.array(num / den, dtype=np.float32)
    return loss, res


def kernel(centers1, features, targets, cls_num_list, conf_mask):
    loss, _ = _run(centers1, features, targets, conf_mask)
    return loss


# revision 12
# speedup vs baseline: 1.3241x; 1.2669x over previous
"""Trainium2 Bass kernel for the BalSCL/SSL balanced supervised-contrastive loss.

Distribution: data-parallel over the 8192 anchor rows, 1024 rows per core on
8 NeuronCores.

The device computes only the O(B2*N) part that needs hardware:
    S_i = sum_c W2[c,i] * EPS[c,i],  EPS[c,i] = sum_{j in c, j!=i} exp(10 f_i.g_j - SH)
and ships the per-row denominator S (one [1,1024] f32 row per core).  The
O(N*D) side terms (per-class feature sums, positive-pair numerator, conf
masking, the final log/mean) are exact closed-form dot products the host
computes in fp64 from the original inputs.

Device pipeline per core (2 i-chunks of 512 x 66 j-tiles):
  - raw logits: bf16 matmuls, [128, 3, 512] PSUM groups (3 j-tiles).
  - self-pair masking: each core's own 8 j-tiles are permuted to positions
    0..7 of its private input copies, so one SPMD program masks the
    diagonal stripes at fixed coordinates (raw -= 30 => exp == 0).
  - exp: split between ScalarE (HW exp -> fp8e4, bias -SH) and VectorE
    (one-op Schraudolph: uint8 bits = round(A*logit + B), bit-cast fp8e4;
    ~3% rms/elem, averages out in the ~83-term class sums).
  - EPS: fp8 DoubleRow matmuls, two j-tiles per instruction (the exp shift
    SH keeps every exp below the TRN fp8e4 Inf threshold of 240).
"""

import os
import sys

sys.path.insert(0, "/opt/trn_rl_repo")

import numpy as np
import ml_dtypes

import concourse.bass as bass  # noqa: F401
import concourse.bacc as bacc
import concourse.tile as tile
from concourse import mybir
from concourse.bass_utils import run_bass_kernel_spmd

F32 = mybir.dt.float32
BF16 = mybir.dt.bfloat16
F8 = mybir.dt.float8e4
U8 = mybir.dt.uint8
BF = ml_dtypes.bfloat16
F8NP = ml_dtypes.float8_e4m3
AF = mybir.ActivationFunctionType
ALU = mybir.AluOpType
DR = mybir.MatmulPerfMode.DoubleRow

B2, C, D = 8192, 100, 128
TEMP = 0.1
N = B2 + C                 # 8292
TJ = 66                    # j-tiles (padded: 8448 = 66*128)
NPAD = TJ * 128
Q = TJ // 2                # 33 j-tile pairs
G = TJ // 3                # 22 exp groups of 3 j-tiles
CORES = 8
R = B2 // CORES            # 1024 rows per core
CH = 512                   # i-chunk width (one fp32 PSUM bank)
CP = 112                   # class dim padded to a 16-byte multiple (fp8 DR lhsT)

SH = 1.25                  # global exp downscale: exp(logit - SH), host adds back
SCH_A = 8.0 / float(np.log(2.0)) * (1.0 / TEMP)   # applied to raw dot
SCH_B = 56.0 - 0.46 - (8.0 / float(np.log(2.0))) * SH

# Pairs whose exp runs on VectorE (the rest on ScalarE).  Diagonal pairs
# (chunk0: 0,1; chunk1: 2,3) must stay on ScalarE.
N_DVE = int(os.environ.get("KB_NDVE", "14"))
# chunk -> pair -> [(slot, 128-col block)] of the diagonal stripes
DIAG = {0: {0: [(0, 0), (1, 1)], 1: [(0, 2), (1, 3)]},
        1: {2: [(0, 0), (1, 1)], 3: [(0, 2), (1, 3)]}}


def _dve_groups(chunk, n):
    forced_act = set(DIAG[chunk].keys())
    elig = [q for q in range(Q) if q not in forced_act]
    step = len(elig) / max(n, 1)
    return {elig[min(int((i + 0.5) * step), len(elig) - 1)] for i in range(n)}


_NC_CACHE = {}

# Combined exp+ln activation-table set: a single ACT_TABLE_LOAD.
_orig_gat = bacc.get_activation_tables


def _gat_combined(arch):
    tabs = _orig_gat(arch)
    out = {}
    for name, funcs in tabs.items():
        if name in ("exp_and_others", "exp_and_friends", "natural_log"):
            out[name] = set()  # keep position (set ids are positional)
        else:
            out[name] = funcs
    return out


def _build_nc():
    bacc.get_activation_tables = _gat_combined
    try:
        return _build_nc_inner()
    finally:
        bacc.get_activation_tables = _orig_gat


def _build_nc_inner():
    nc = bacc.Bacc()

    fTcd = nc.dram_tensor("fTc", [D, R], BF16, kind="ExternalInput")
    fTgd = nc.dram_tensor("fTg", [D, NPAD], BF16, kind="ExternalInput")
    dmaskd = nc.dram_tensor("dmask", [128, 128], BF16, kind="ExternalInput")
    TAgd = nc.dram_tensor("TAg8", [128, TJ, CP], F8, kind="ExternalInput")
    W2d = nc.dram_tensor("W2", [C, R], BF16, kind="ExternalInput")
    srowd = nc.dram_tensor("srow", [1, R], F32, kind="ExternalOutput")

    dve_sets = [_dve_groups(0, N_DVE), _dve_groups(1, N_DVE)]

    with tile.TileContext(nc) as tc:
        with (
            tc.tile_pool(name="consts", bufs=1) as cp,
            tc.tile_pool(name="rawp", bufs=3, space="PSUM") as rp,
            tc.tile_pool(name="epsp", bufs=1, space="PSUM") as pp,
            tc.tile_pool(name="smp", bufs=1, space="PSUM") as sp,
        ):
            # ---------------- input DMA (ordered by first use) ----------
            s_fTc = cp.tile([D, R], BF16)
            nc.sync.dma_start(out=s_fTc, in_=fTcd[:])
            s_fTg = cp.tile([D, NPAD], BF16)
            s_TAg = cp.tile([128, TJ, CP], F8)
            nc.sync.dma_start(out=s_fTg[:, 0:1536], in_=fTgd[:, 0:1536])
            s_dmask = cp.tile([128, 128], BF16)
            nc.sync.dma_start(out=s_dmask, in_=dmaskd[:])
            nc.sync.dma_start(out=s_TAg[:, 0:12, :], in_=TAgd[:, 0:12, :])
            nc.sync.dma_start(out=s_fTg[:, 1536:4608], in_=fTgd[:, 1536:4608])
            nc.sync.dma_start(out=s_TAg[:, 12:36, :], in_=TAgd[:, 12:36, :])
            nc.sync.dma_start(out=s_fTg[:, 4608:NPAD], in_=fTgd[:, 4608:NPAD])
            nc.sync.dma_start(out=s_TAg[:, 36:TJ, :], in_=TAgd[:, 36:TJ, :])
            s_W2 = cp.tile([C, R], BF16)
            nc.sync.dma_start(out=s_W2, in_=W2d[:])

            s_negsh = cp.tile([128, 1], F32)
            nc.gpsimd.memset(s_negsh, -SH)
            s_ones_bf = cp.tile([128, 1], BF16)
            nc.gpsimd.memset(s_ones_bf, 1.0)

            # trigger the ACT table load in the DMA window
            tabt = cp.tile([1, 16], F32)
            nc.scalar.activation(
                out=tabt, in_=s_negsh[0:1, :].to_broadcast((1, 16)), func=AF.Exp
            )

            s_ex = [
                cp.tile([128, TJ, CH], F8, name="ex0"),
                cp.tile([128, TJ, CH], F8, name="ex1"),
            ]
            s_srow = cp.tile([1, R], F32)
            W2E = [None, None]

            # ---------------- per-chunk pipeline ------------------------
            def chunk_body(c, extras):
                i0 = c * CH
                dve_set = dve_sets[c]
                EPS = pp.tile([CP, CH], F32, name=f"EPS{c}", tag="EPS")
                for q in range(Q):
                    rawT = rp.tile([128, 2, CH], F32, name="rawT", tag="raw")
                    for s in (0, 1):
                        t = 2 * q + s
                        nc.tensor.matmul(
                            rawT[:, s, :],
                            lhsT=s_fTg[:, 128 * t : 128 * (t + 1)],
                            rhs=s_fTc[:, i0 : i0 + CH],
                            start=True, stop=True,
                        )
                    for s, b in DIAG[c].get(q, ()):
                        nc.vector.scalar_tensor_tensor(
                            out=rawT[:, s, 128 * b : 128 * (b + 1)],
                            in0=s_dmask, scalar=-30.0,
                            in1=rawT[:, s, 128 * b : 128 * (b + 1)],
                            op0=ALU.mult, op1=ALU.add,
                        )
                    exd = s_ex[c][:, 2 * q : 2 * q + 2, :]
                    if q in dve_set:
                        nc.vector.tensor_scalar(
                            exd.bitcast(U8), rawT[:, :, :],
                            SCH_A, SCH_B, op0=ALU.mult, op1=ALU.add,
                        )
                    else:
                        nc.scalar.activation(
                            out=exd, in_=rawT[:, :, :], func=AF.Exp,
                            scale=1.0 / TEMP, bias=s_negsh,
                        )
                    nc.tensor.matmul(
                        EPS,
                        lhsT=s_TAg[:, 2 * q : 2 * q + 2, :],
                        rhs=exd,
                        start=(q == 0), stop=(q == Q - 1), perf_mode=DR,
                    )
                    for fn in extras.pop(q, ()):
                        fn()
                for fns in extras.values():
                    for fn in fns:
                        fn()
                return EPS

            def mk_w2e(c, EPS):
                def go():
                    i0 = c * CH
                    w = cp.tile([C, CH], BF16, name=f"W2E{c}", tag=f"W2E{c}")
                    nc.vector.tensor_mul(w, EPS[0:C, :], s_W2[:, i0 : i0 + CH])
                    W2E[c] = w
                return go

            def mk_srow(c):
                def go():
                    i0 = c * CH
                    SrowPS = sp.tile([1, CH], F32, name=f"SrowPS{c}", tag="sm")
                    nc.tensor.matmul(
                        SrowPS, lhsT=s_ones_bf[0:C, :], rhs=W2E[c],
                        start=True, stop=True,
                    )
                    nc.vector.tensor_copy(s_srow[:, i0 : i0 + CH], SrowPS)
                return go

            EPS0 = chunk_body(0, {})
            mk_w2e(0, EPS0)()
            EPS1 = chunk_body(1, {2: [mk_srow(0)]})

            # ---------------- tail ----------------
            mk_w2e(1, EPS1)()
            mk_srow(1)()
            nc.sync.dma_start(out=srowd[:], in_=s_srow)

    nc.finalize()
    return nc


def _get_nc():
    if "nc" not in _NC_CACHE:
        _NC_CACHE["nc"] = _build_nc()
    return _NC_CACHE["nc"]


def _prep_inputs(centers1, features, targets):
    f32 = np.float32
    feats_all = np.concatenate([features, centers1], axis=0)  # [N, D]
    fa_pad = np.zeros((NPAD, D), dtype=f32)
    fa_pad[:N] = feats_all
    TA_pad = np.zeros((NPAD, C), dtype=f32)
    TA_pad[:B2] = targets
    TA_pad[B2:N] = np.eye(C, dtype=f32)

    fa_t = fa_pad.reshape(TJ, 128, D)
    TA_t = TA_pad.reshape(TJ, 128, C)

    cc = targets.sum(axis=0, dtype=np.float64) + 1.0  # [C]
    safe = cc > 1.5
    dcls = np.where(safe, 1.0 / np.maximum(cc - 1.0, 1.0) - 1.0 / cc, 0.0)
    invc = 1.0 / cc

    dmask_np = np.eye(128, dtype=f32).astype(BF)

    in_maps = []
    for k in range(CORES):
        rows = slice(k * R, (k + 1) * R)
        # permute j-tiles: this core's own 8 tiles go to positions 0..7
        own = list(range(8 * k, 8 * k + 8))
        perm = own + [t for t in range(TJ) if t not in own]
        fa_k = fa_t[perm]                      # [TJ, 128, D]
        TA_k = TA_t[perm]                      # [TJ, 128, C]

        fTg_np = np.ascontiguousarray(
            fa_k.transpose(2, 0, 1).reshape(D, NPAD)
        ).astype(BF)
        TA_kp = np.zeros((TJ, 128, CP), dtype=f32)
        TA_kp[:, :, :C] = TA_k
        TAg8_np = np.ascontiguousarray(TA_kp.transpose(1, 0, 2)).astype(F8NP)
        fTc_np = np.ascontiguousarray(features[rows].T).astype(BF)
        tTp_f32 = np.ascontiguousarray(targets[rows].T, dtype=f32)  # [C, R]
        W2_np = (dcls[:, None] * tTp_f32 + invc[:, None]).astype(BF)
        in_maps.append(
            {
                "fTc": fTc_np,
                "fTg": fTg_np,
                "dmask": dmask_np,
                "TAg8": TAg8_np,
                "W2": W2_np,
            }
        )
    return in_maps


def _run(centers1, features, targets, conf_mask, trace=False, trace_cores=None):
    f32 = np.float32
    features = np.ascontiguousarray(features, dtype=f32)
    centers1 = np.ascontiguousarray(centers1, dtype=f32).reshape(-1, D)
    targets = np.ascontiguousarray(targets, dtype=f32)
    conf = np.ascontiguousarray(conf_mask, dtype=np.float64)

    in_maps = _prep_inputs(centers1, features, targets)
    nc = _get_nc()
    kwargs = {}
    if trace:
        # NTFF profiling under axon: shim the (absent) antenv.axon_hooks
        # module and skip the artifact bucket upload.
        import types
        import concourse.bass_utils as bass_utils

        if "antenv.axon_hooks" not in sys.modules:
            mod = types.ModuleType("antenv.axon_hooks")
            mod._hook = None

            def set_axon_ntff_profile_hook(h):
                mod._hook = h

            def get_axon_ntff_profile_hook():
                return mod._hook

            mod.set_axon_ntff_profile_hook = set_axon_ntff_profile_hook
            mod.get_axon_ntff_profile_hook = get_axon_ntff_profile_hook
            sys.modules["antenv.axon_hooks"] = mod
            from trn_agent_boot.trn_boot import _ntff_profile_via_ctypes

            set_axon_ntff_profile_hook(
                _ntff_profile_via_ctypes("/opt/axon/libaxon_pjrt.so")
            )
        bass_utils.upload_artifacts = lambda tmpdir: "local://" + tmpdir
        kwargs = {"trace": True}
        if trace_cores is not None:
            kwargs["trace_cores"] = trace_cores
    res = run_bass_kernel_spmd(nc, in_maps, core_ids=list(range(CORES)), **kwargs)

    # host-side closed-form pieces (fp64, exact)
    S = np.concatenate([r["srow"][0] for r in res.results]).astype(np.float64)
    labs = targets.argmax(axis=1)
    cc = targets.sum(axis=0, dtype=np.float64) + 1.0
    gsum = targets.T.astype(np.float64) @ features.astype(np.float64)
    gsum += centers1.astype(np.float64)                      # [C, D]
    gath = gsum[labs]                                        # [B2, D]
    f64 = features.astype(np.float64)
    sm_raw = (f64 * gath).sum(axis=1) - (f64 * f64).sum(axis=1)
    SmT = (1.0 / TEMP) * sm_raw / (cc[labs] - 1.0)
    lnS = np.log(np.maximum(S, 1e-30)) + SH
    num = (conf * (lnS - SmT)).sum()
    den = conf.sum()
    loss = np.array(num / den, dtype=np.float32)
    return loss, res


def kernel(centers1, features, targets, cls_num_list, conf_mask):
    loss, _ = _run(centers1, features, targets, conf_mask)
    return loss


# revision 13
# speedup vs baseline: 1.3500x; 1.0196x over previous
"""Trainium2 Bass kernel for the BalSCL/SSL balanced supervised-contrastive loss.

Distribution: data-parallel over the 8192 anchor rows, 1024 rows per core on
8 NeuronCores.

The device computes only the O(B2*N) part that needs hardware:
    S_i = sum_c W2[c,i] * EPS[c,i],  EPS[c,i] = sum_{j in c, j!=i} exp(10 f_i.g_j - SH)
and ships the per-row denominator S (one [1,1024] f32 row per core).  The
O(N*D) side terms (per-class feature sums, positive-pair numerator, conf
masking, the final log/mean) are exact closed-form dot products the host
computes in fp64 from the original inputs.

Device pipeline per core (2 i-chunks of 512 x 66 j-tiles):
  - raw logits: bf16 matmuls, [128, 3, 512] PSUM groups (3 j-tiles).
  - self-pair masking: each core's own 8 j-tiles are permuted to positions
    0..7 of its private input copies, so one SPMD program masks the
    diagonal stripes at fixed coordinates (raw -= 30 => exp == 0).
  - exp: split between ScalarE (HW exp -> fp8e4, bias -SH) and VectorE
    (one-op Schraudolph: uint8 bits = round(A*logit + B), bit-cast fp8e4;
    ~3% rms/elem, averages out in the ~83-term class sums).
  - EPS: fp8 DoubleRow matmuls, two j-tiles per instruction (the exp shift
    SH keeps every exp below the TRN fp8e4 Inf threshold of 240).
"""

import os
import sys

sys.path.insert(0, "/opt/trn_rl_repo")

import numpy as np
import ml_dtypes

import concourse.bass as bass  # noqa: F401
import concourse.bacc as bacc
import concourse.tile as tile
from concourse import mybir
from concourse.bass_utils import run_bass_kernel_spmd

F32 = mybir.dt.float32
BF16 = mybir.dt.bfloat16
F8 = mybir.dt.float8e4
U8 = mybir.dt.uint8
BF = ml_dtypes.bfloat16
F8NP = ml_dtypes.float8_e4m3
AF = mybir.ActivationFunctionType
ALU = mybir.AluOpType
DR = mybir.MatmulPerfMode.DoubleRow

B2, C, D = 8192, 100, 128
TEMP = 0.1
N = B2 + C                 # 8292
TJ = 66                    # j-tiles (padded: 8448 = 66*128)
NPAD = TJ * 128
Q = TJ // 2                # 33 j-tile pairs
G = TJ // 3                # 22 exp groups of 3 j-tiles
CORES = 8
R = B2 // CORES            # 1024 rows per core
CH = 512                   # i-chunk width (one fp32 PSUM bank)
CP = 112                   # class dim padded to a 16-byte multiple (fp8 DR lhsT)

SH = 1.25                  # global exp downscale: exp(logit - SH), host adds back
SCH_A = 8.0 / float(np.log(2.0)) * (1.0 / TEMP)   # applied to raw dot
SCH_B = 56.0 - 0.46 - (8.0 / float(np.log(2.0))) * SH

# Pairs whose exp runs on VectorE (the rest on ScalarE).  Diagonal pairs
# (chunk0: 0,1; chunk1: 2,3) must stay on ScalarE.
N_DVE = int(os.environ.get("KB_NDVE", "15"))
# chunk -> pair -> [(slot, 128-col block)] of the diagonal stripes
DIAG = {0: {0: [(0, 0), (1, 1)], 1: [(0, 2), (1, 3)]},
        1: {2: [(0, 0), (1, 1)], 3: [(0, 2), (1, 3)]}}


def _dve_groups(chunk, n):
    forced_act = set(DIAG[chunk].keys())
    elig = [q for q in range(Q) if q not in forced_act]
    step = len(elig) / max(n, 1)
    return {elig[min(int((i + 0.5) * step), len(elig) - 1)] for i in range(n)}


_NC_CACHE = {}

# Combined exp+ln activation-table set: a single ACT_TABLE_LOAD.
_orig_gat = bacc.get_activation_tables


def _gat_combined(arch):
    tabs = _orig_gat(arch)
    out = {}
    for name, funcs in tabs.items():
        if name in ("exp_and_others", "exp_and_friends", "natural_log"):
            out[name] = set()  # keep position (set ids are positional)
        else:
            out[name] = funcs
    return out


def _build_nc():
    bacc.get_activation_tables = _gat_combined
    try:
        return _build_nc_inner()
    finally:
        bacc.get_activation_tables = _orig_gat


def _build_nc_inner():
    nc = bacc.Bacc()

    fTcd = nc.dram_tensor("fTc", [D, R], BF16, kind="ExternalInput")
    fTgd = nc.dram_tensor("fTg", [D, NPAD], BF16, kind="ExternalInput")
    dmaskd = nc.dram_tensor("dmask", [128, 128], BF16, kind="ExternalInput")
    TAgd = nc.dram_tensor("TAg8", [128, TJ, CP], F8, kind="ExternalInput")
    W2d = nc.dram_tensor("W2", [C, R], BF16, kind="ExternalInput")
    srowd = nc.dram_tensor("srow", [1, R], F32, kind="ExternalOutput")

    dve_sets = [_dve_groups(0, N_DVE), _dve_groups(1, N_DVE)]

    with tile.TileContext(nc) as tc:
        with (
            tc.tile_pool(name="consts", bufs=1) as cp,
            tc.tile_pool(name="rawp", bufs=3, space="PSUM") as rp,
            tc.tile_pool(name="epsp", bufs=1, space="PSUM") as pp,
            tc.tile_pool(name="smp", bufs=1, space="PSUM") as sp,
        ):
            # ---------------- input DMA (ordered by first use) ----------
            s_fTc = cp.tile([D, R], BF16)
            nc.sync.dma_start(out=s_fTc, in_=fTcd[:])
            s_fTg = cp.tile([D, NPAD], BF16)
            s_TAg = cp.tile([128, TJ, CP], F8)
            nc.sync.dma_start(out=s_fTg[:, 0:1536], in_=fTgd[:, 0:1536])
            s_dmask = cp.tile([128, 128], BF16)
            nc.sync.dma_start(out=s_dmask, in_=dmaskd[:])
            nc.sync.dma_start(out=s_TAg[:, 0:12, :], in_=TAgd[:, 0:12, :])
            nc.sync.dma_start(out=s_fTg[:, 1536:4608], in_=fTgd[:, 1536:4608])
            nc.sync.dma_start(out=s_TAg[:, 12:36, :], in_=TAgd[:, 12:36, :])
            nc.sync.dma_start(out=s_fTg[:, 4608:NPAD], in_=fTgd[:, 4608:NPAD])
            nc.sync.dma_start(out=s_TAg[:, 36:TJ, :], in_=TAgd[:, 36:TJ, :])
            s_W2 = cp.tile([C, R], BF16)
            nc.sync.dma_start(out=s_W2, in_=W2d[:])

            s_negsh = cp.tile([128, 1], F32)
            nc.gpsimd.memset(s_negsh, -SH)
            s_ones_bf = cp.tile([128, 1], BF16)
            nc.gpsimd.memset(s_ones_bf, 1.0)

            # trigger the ACT table load in the DMA window
            tabt = cp.tile([1, 16], F32)
            nc.scalar.activation(
                out=tabt, in_=s_negsh[0:1, :].to_broadcast((1, 16)), func=AF.Exp
            )

            # HAM warm-up: dense matmuls gated only on the first DMA (fTc)
            warmT = rp.tile([128, 2, CH], F32, name="warmT", tag="raw")
            for w in range(6):
                nc.tensor.matmul(
                    warmT[:, w % 2, :], lhsT=s_fTc[:, 0:128],
                    rhs=s_fTc[:, 0:CH], start=True, stop=True,
                )

            s_ex = [
                cp.tile([128, TJ, CH], F8, name="ex0"),
                cp.tile([128, TJ, CH], F8, name="ex1"),
            ]
            s_srow = cp.tile([1, R], F32)
            W2E = [None, None]

            # ---------------- per-chunk pipeline ------------------------
            def chunk_body(c, extras):
                i0 = c * CH
                dve_set = dve_sets[c]
                EPS = pp.tile([CP, CH], F32, name=f"EPS{c}", tag="EPS")
                for q in range(Q):
                    rawT = rp.tile([128, 2, CH], F32, name="rawT", tag="raw")
                    for s in (0, 1):
                        t = 2 * q + s
                        nc.tensor.matmul(
                            rawT[:, s, :],
                            lhsT=s_fTg[:, 128 * t : 128 * (t + 1)],
                            rhs=s_fTc[:, i0 : i0 + CH],
                            start=True, stop=True,
                        )
                    for s, b in DIAG[c].get(q, ()):
                        nc.vector.scalar_tensor_tensor(
                            out=rawT[:, s, 128 * b : 128 * (b + 1)],
                            in0=s_dmask, scalar=-30.0,
                            in1=rawT[:, s, 128 * b : 128 * (b + 1)],
                            op0=ALU.mult, op1=ALU.add,
                        )
                    exd = s_ex[c][:, 2 * q : 2 * q + 2, :]
                    if q in dve_set:
                        nc.vector.tensor_scalar(
                            exd.bitcast(U8), rawT[:, :, :],
                            SCH_A, SCH_B, op0=ALU.mult, op1=ALU.add,
                        )
                    else:
                        nc.scalar.activation(
                            out=exd, in_=rawT[:, :, :], func=AF.Exp,
                            scale=1.0 / TEMP, bias=s_negsh,
                        )
                    nc.tensor.matmul(
                        EPS,
                        lhsT=s_TAg[:, 2 * q : 2 * q + 2, :],
                        rhs=exd,
                        start=(q == 0), stop=(q == Q - 1), perf_mode=DR,
                    )
                    for fn in extras.pop(q, ()):
                        fn()
                for fns in extras.values():
                    for fn in fns:
                        fn()
                return EPS

            def mk_w2e(c, EPS):
                def go():
                    i0 = c * CH
                    w = cp.tile([C, CH], BF16, name=f"W2E{c}", tag=f"W2E{c}")
                    nc.vector.tensor_mul(w, EPS[0:C, :], s_W2[:, i0 : i0 + CH])
                    W2E[c] = w
                return go

            def mk_srow(c):
                def go():
                    i0 = c * CH
                    SrowPS = sp.tile([1, CH], F32, name=f"SrowPS{c}", tag="sm")
                    nc.tensor.matmul(
                        SrowPS, lhsT=s_ones_bf[0:C, :], rhs=W2E[c],
                        start=True, stop=True,
                    )
                    nc.vector.tensor_copy(s_srow[:, i0 : i0 + CH], SrowPS)
                    nc.sync.dma_start(
                        out=srowd[:, i0 : i0 + CH], in_=s_srow[:, i0 : i0 + CH]
                    )
                return go

            EPS0 = chunk_body(0, {})
            mk_w2e(0, EPS0)()
            EPS1 = chunk_body(1, {2: [mk_srow(0)]})

            # ---------------- tail ----------------
            mk_w2e(1, EPS1)()
            mk_srow(1)()

    nc.finalize()
    return nc


def _get_nc():
    if "nc" not in _NC_CACHE:
        _NC_CACHE["nc"] = _build_nc()
    return _NC_CACHE["nc"]


def _prep_inputs(centers1, features, targets):
    f32 = np.float32
    feats_all = np.concatenate([features, centers1], axis=0)  # [N, D]
    fa_pad = np.zeros((NPAD, D), dtype=f32)
    fa_pad[:N] = feats_all
    TA_pad = np.zeros((NPAD, C), dtype=f32)
    TA_pad[:B2] = targets
    TA_pad[B2:N] = np.eye(C, dtype=f32)

    fa_t = fa_pad.reshape(TJ, 128, D)
    TA_t = TA_pad.reshape(TJ, 128, C)

    cc = targets.sum(axis=0, dtype=np.float64) + 1.0  # [C]
    safe = cc > 1.5
    dcls = np.where(safe, 1.0 / np.maximum(cc - 1.0, 1.0) - 1.0 / cc, 0.0)
    invc = 1.0 / cc

    dmask_np = np.eye(128, dtype=f32).astype(BF)

    in_maps = []
    for k in range(CORES):
        rows = slice(k * R, (k + 1) * R)
        # permute j-tiles: this core's own 8 tiles go to positions 0..7
        own = list(range(8 * k, 8 * k + 8))
        perm = own + [t for t in range(TJ) if t not in own]
        fa_k = fa_t[perm]                      # [TJ, 128, D]
        TA_k = TA_t[perm]                      # [TJ, 128, C]

        fTg_np = np.ascontiguousarray(
            fa_k.transpose(2, 0, 1).reshape(D, NPAD)
        ).astype(BF)
        TA_kp = np.zeros((TJ, 128, CP), dtype=f32)
        TA_kp[:, :, :C] = TA_k
        TAg8_np = np.ascontiguousarray(TA_kp.transpose(1, 0, 2)).astype(F8NP)
        fTc_np = np.ascontiguousarray(features[rows].T).astype(BF)
        tTp_f32 = np.ascontiguousarray(targets[rows].T, dtype=f32)  # [C, R]
        W2_np = (dcls[:, None] * tTp_f32 + invc[:, None]).astype(BF)
        in_maps.append(
            {
                "fTc": fTc_np,
                "fTg": fTg_np,
                "dmask": dmask_np,
                "TAg8": TAg8_np,
                "W2": W2_np,
            }
        )
    return in_maps


def _run(centers1, features, targets, conf_mask, trace=False, trace_cores=None):
    f32 = np.float32
    features = np.ascontiguousarray(features, dtype=f32)
    centers1 = np.ascontiguousarray(centers1, dtype=f32).reshape(-1, D)
    targets = np.ascontiguousarray(targets, dtype=f32)
    conf = np.ascontiguousarray(conf_mask, dtype=np.float64)

    in_maps = _prep_inputs(centers1, features, targets)
    nc = _get_nc()
    kwargs = {}
    if trace:
        # NTFF profiling under axon: shim the (absent) antenv.axon_hooks
        # module and skip the artifact bucket upload.
        import types
        import concourse.bass_utils as bass_utils

        if "antenv.axon_hooks" not in sys.modules:
            mod = types.ModuleType("antenv.axon_hooks")
            mod._hook = None

            def set_axon_ntff_profile_hook(h):
                mod._hook = h

            def get_axon_ntff_profile_hook():
                return mod._hook

            mod.set_axon_ntff_profile_hook = set_axon_ntff_profile_hook
            mod.get_axon_ntff_profile_hook = get_axon_ntff_profile_hook
            sys.modules["antenv.axon_hooks"] = mod
            from trn_agent_boot.trn_boot import _ntff_profile_via_ctypes

            set_axon_ntff_profile_hook(
                _ntff_profile_via_ctypes("/opt/axon/libaxon_pjrt.so")
            )
        bass_utils.upload_artifacts = lambda tmpdir: "local://" + tmpdir
        kwargs = {"trace": True}
        if trace_cores is not None:
            kwargs["trace_cores"] = trace_cores
    res = run_bass_kernel_spmd(nc, in_maps, core_ids=list(range(CORES)), **kwargs)

    # host-side closed-form pieces (fp64, exact)
    S = np.concatenate([r["srow"][0] for r in res.results]).astype(np.float64)
    labs = targets.argmax(axis=1)
    cc = targets.sum(axis=0, dtype=np.float64) + 1.0
    gsum = targets.T.astype(np.float64) @ features.astype(np.float64)
    gsum += centers1.astype(np.float64)                      # [C, D]
    gath = gsum[labs]                                        # [B2, D]
    f64 = features.astype(np.float64)
    sm_raw = (f64 * gath).sum(axis=1) - (f64 * f64).sum(axis=1)
    SmT = (1.0 / TEMP) * sm_raw / (cc[labs] - 1.0)
    lnS = np.log(np.maximum(S, 1e-30)) + SH
    num = (conf * (lnS - SmT)).sum()
    den = conf.sum()
    loss = np.array(num / den, dtype=np.float32)
    return loss, res


def kernel(centers1, features, targets, cls_num_list, conf_mask):
    loss, _ = _run(centers1, features, targets, conf_mask)
    return loss


# revision 14
# speedup vs baseline: 1.3524x; 1.0017x over previous
"""Trainium2 Bass kernel for the BalSCL/SSL balanced supervised-contrastive loss.

Distribution: data-parallel over the 8192 anchor rows, 1024 rows per core on
8 NeuronCores.

The device computes only the O(B2*N) part that needs hardware:
    S_i = sum_c W2[c,i] * EPS[c,i],  EPS[c,i] = sum_{j in c, j!=i} exp(10 f_i.g_j - SH)
and ships the per-row denominator S (one [1,1024] f32 row per core).  The
O(N*D) side terms (per-class feature sums, positive-pair numerator, conf
masking, the final log/mean) are exact closed-form dot products the host
computes in fp64 from the original inputs.

Device pipeline per core (2 i-chunks of 512 x 66 j-tiles):
  - raw logits: bf16 matmuls, [128, 3, 512] PSUM groups (3 j-tiles).
  - self-pair masking: each core's own 8 j-tiles are permuted to positions
    0..7 of its private input copies, so one SPMD program masks the
    diagonal stripes at fixed coordinates (raw -= 30 => exp == 0).
  - exp: split between ScalarE (HW exp -> fp8e4, bias -SH) and VectorE
    (one-op Schraudolph: uint8 bits = round(A*logit + B), bit-cast fp8e4;
    ~3% rms/elem, averages out in the ~83-term class sums).
  - EPS: fp8 DoubleRow matmuls, two j-tiles per instruction (the exp shift
    SH keeps every exp below the TRN fp8e4 Inf threshold of 240).
"""

import os
import sys

sys.path.insert(0, "/opt/trn_rl_repo")

import numpy as np
import ml_dtypes

import concourse.bass as bass  # noqa: F401
import concourse.bacc as bacc
import concourse.tile as tile
from concourse import mybir
from concourse.bass_utils import run_bass_kernel_spmd

F32 = mybir.dt.float32
BF16 = mybir.dt.bfloat16
F8 = mybir.dt.float8e4
U8 = mybir.dt.uint8
BF = ml_dtypes.bfloat16
F8NP = ml_dtypes.float8_e4m3
AF = mybir.ActivationFunctionType
ALU = mybir.AluOpType
DR = mybir.MatmulPerfMode.DoubleRow

B2, C, D = 8192, 100, 128
TEMP = 0.1
N = B2 + C                 # 8292
TJ = 66                    # j-tiles (padded: 8448 = 66*128)
NPAD = TJ * 128
Q = TJ // 2                # 33 j-tile pairs
G = TJ // 3                # 22 exp groups of 3 j-tiles
CORES = 8
R = B2 // CORES            # 1024 rows per core
CH = 512                   # i-chunk width (one fp32 PSUM bank)
CP = 112                   # class dim padded to a 16-byte multiple (fp8 DR lhsT)

SH = 1.25                  # global exp downscale: exp(logit - SH), host adds back
SCH_A = 8.0 / float(np.log(2.0)) * (1.0 / TEMP)   # applied to raw dot
SCH_B = 56.0 - 0.46 - (8.0 / float(np.log(2.0))) * SH

# Pairs whose exp runs on VectorE (the rest on ScalarE).  Diagonal pairs
# (chunk0: 0,1; chunk1: 2,3) must stay on ScalarE.
N_DVE = int(os.environ.get("KB_NDVE", "15"))
# chunk -> pair -> [(slot, 128-col block)] of the diagonal stripes
DIAG = {0: {0: [(0, 0), (1, 1)], 1: [(0, 2), (1, 3)]},
        1: {2: [(0, 0), (1, 1)], 3: [(0, 2), (1, 3)]}}


def _dve_groups(chunk, n):
    forced_act = set(DIAG[chunk].keys())
    elig = [q for q in range(Q) if q not in forced_act]
    step = len(elig) / max(n, 1)
    return {elig[min(int((i + 0.5) * step), len(elig) - 1)] for i in range(n)}


_NC_CACHE = {}

# Combined exp+ln activation-table set: a single ACT_TABLE_LOAD.
_orig_gat = bacc.get_activation_tables


def _gat_combined(arch):
    tabs = _orig_gat(arch)
    out = {}
    for name, funcs in tabs.items():
        if name in ("exp_and_others", "exp_and_friends", "natural_log"):
            out[name] = set()  # keep position (set ids are positional)
        else:
            out[name] = funcs
    return out


def _build_nc():
    bacc.get_activation_tables = _gat_combined
    try:
        return _build_nc_inner()
    finally:
        bacc.get_activation_tables = _orig_gat


def _build_nc_inner():
    nc = bacc.Bacc()

    fTcd = nc.dram_tensor("fTc", [D, R], BF16, kind="ExternalInput")
    fTgd = nc.dram_tensor("fTg", [D, NPAD], BF16, kind="ExternalInput")
    dmaskd = nc.dram_tensor("dmask", [128, 128], BF16, kind="ExternalInput")
    TAgd = nc.dram_tensor("TAg8", [128, TJ, CP], F8, kind="ExternalInput")
    W2d = nc.dram_tensor("W2", [C, R], BF16, kind="ExternalInput")
    srowd = nc.dram_tensor("srow", [1, R], F32, kind="ExternalOutput")

    dve_sets = [_dve_groups(0, N_DVE), _dve_groups(1, N_DVE)]

    with tile.TileContext(nc) as tc:
        with (
            tc.tile_pool(name="consts", bufs=1) as cp,
            tc.tile_pool(name="rawp", bufs=3, space="PSUM") as rp,
            tc.tile_pool(name="epsp", bufs=1, space="PSUM") as pp,
            tc.tile_pool(name="smp", bufs=1, space="PSUM") as sp,
        ):
            # ---------------- input DMA (ordered by first use) ----------
            s_fTc = cp.tile([D, R], BF16)
            nc.sync.dma_start(out=s_fTc[:, 0:CH], in_=fTcd[:, 0:CH])
            s_fTg = cp.tile([D, NPAD], BF16)
            s_TAg = cp.tile([128, TJ, CP], F8)
            nc.sync.dma_start(out=s_fTg[:, 0:1536], in_=fTgd[:, 0:1536])
            s_dmask = cp.tile([128, 128], BF16)
            nc.sync.dma_start(out=s_dmask, in_=dmaskd[:])
            nc.sync.dma_start(out=s_fTc[:, CH:R], in_=fTcd[:, CH:R])
            nc.sync.dma_start(out=s_TAg[:, 0:12, :], in_=TAgd[:, 0:12, :])
            nc.sync.dma_start(out=s_fTg[:, 1536:4608], in_=fTgd[:, 1536:4608])
            nc.sync.dma_start(out=s_TAg[:, 12:36, :], in_=TAgd[:, 12:36, :])
            nc.sync.dma_start(out=s_fTg[:, 4608:NPAD], in_=fTgd[:, 4608:NPAD])
            nc.sync.dma_start(out=s_TAg[:, 36:TJ, :], in_=TAgd[:, 36:TJ, :])
            s_W2 = cp.tile([C, R], BF16)
            nc.sync.dma_start(out=s_W2, in_=W2d[:])

            s_negsh = cp.tile([128, 1], F32)
            nc.gpsimd.memset(s_negsh, -SH)
            s_ones_bf = cp.tile([128, 1], BF16)
            nc.gpsimd.memset(s_ones_bf, 1.0)

            # trigger the ACT table load in the DMA window
            tabt = cp.tile([1, 16], F32)
            nc.scalar.activation(
                out=tabt, in_=s_negsh[0:1, :].to_broadcast((1, 16)), func=AF.Exp
            )

            # HAM warm-up: dense matmuls gated only on the first DMA (fTc)
            warmT = rp.tile([128, 2, CH], F32, name="warmT", tag="raw")
            for w in range(3):
                nc.tensor.matmul(
                    warmT[:, w % 2, :], lhsT=s_fTc[:, 0:128],
                    rhs=s_fTc[:, 0:CH], start=True, stop=True,
                )

            s_ex = [
                cp.tile([128, TJ, CH], F8, name="ex0"),
                cp.tile([128, TJ, CH], F8, name="ex1"),
            ]
            s_srow = cp.tile([1, R], F32)
            W2E = [None, None]

            # ---------------- per-chunk pipeline ------------------------
            def chunk_body(c, extras):
                i0 = c * CH
                dve_set = dve_sets[c]
                EPS = pp.tile([CP, CH], F32, name=f"EPS{c}", tag="EPS")
                for q in range(Q):
                    rawT = rp.tile([128, 2, CH], F32, name="rawT", tag="raw")
                    for s in (0, 1):
                        t = 2 * q + s
                        nc.tensor.matmul(
                            rawT[:, s, :],
                            lhsT=s_fTg[:, 128 * t : 128 * (t + 1)],
                            rhs=s_fTc[:, i0 : i0 + CH],
                            start=True, stop=True,
                        )
                    for s, b in DIAG[c].get(q, ()):
                        nc.vector.scalar_tensor_tensor(
                            out=rawT[:, s, 128 * b : 128 * (b + 1)],
                            in0=s_dmask, scalar=-30.0,
                            in1=rawT[:, s, 128 * b : 128 * (b + 1)],
                            op0=ALU.mult, op1=ALU.add,
                        )
                    exd = s_ex[c][:, 2 * q : 2 * q + 2, :]
                    if q in dve_set:
                        nc.vector.tensor_scalar(
                            exd.bitcast(U8), rawT[:, :, :],
                            SCH_A, SCH_B, op0=ALU.mult, op1=ALU.add,
                        )
                    else:
                        nc.scalar.activation(
                            out=exd, in_=rawT[:, :, :], func=AF.Exp,
                            scale=1.0 / TEMP, bias=s_negsh,
                        )
                    nc.tensor.matmul(
                        EPS,
                        lhsT=s_TAg[:, 2 * q : 2 * q + 2, :],
                        rhs=exd,
                        start=(q == 0), stop=(q == Q - 1), perf_mode=DR,
                    )
                    for fn in extras.pop(q, ()):
                        fn()
                for fns in extras.values():
                    for fn in fns:
                        fn()
                return EPS

            def mk_w2e(c, EPS):
                def go():
                    i0 = c * CH
                    w = cp.tile([C, CH], BF16, name=f"W2E{c}", tag=f"W2E{c}")
                    nc.vector.tensor_mul(w, EPS[0:C, :], s_W2[:, i0 : i0 + CH])
                    W2E[c] = w
                return go

            def mk_srow(c):
                def go():
                    i0 = c * CH
                    SrowPS = sp.tile([1, CH], F32, name=f"SrowPS{c}", tag="sm")
                    nc.tensor.matmul(
                        SrowPS, lhsT=s_ones_bf[0:C, :], rhs=W2E[c],
                        start=True, stop=True,
                    )
                    nc.vector.tensor_copy(s_srow[:, i0 : i0 + CH], SrowPS)
                    nc.sync.dma_start(
                        out=srowd[:, i0 : i0 + CH], in_=s_srow[:, i0 : i0 + CH]
                    )
                return go

            EPS0 = chunk_body(0, {})
            mk_w2e(0, EPS0)()
            EPS1 = chunk_body(1, {2: [mk_srow(0)]})

            # ---------------- tail ----------------
            mk_w2e(1, EPS1)()
            mk_srow(1)()

    nc.finalize()
    return nc


def _get_nc():
    if "nc" not in _NC_CACHE:
        _NC_CACHE["nc"] = _build_nc()
    return _NC_CACHE["nc"]


def _prep_inputs(centers1, features, targets):
    f32 = np.float32
    feats_all = np.concatenate([features, centers1], axis=0)  # [N, D]
    fa_pad = np.zeros((NPAD, D), dtype=f32)
    fa_pad[:N] = feats_all
    TA_pad = np.zeros((NPAD, C), dtype=f32)
    TA_pad[:B2] = targets
    TA_pad[B2:N] = np.eye(C, dtype=f32)

    fa_t = fa_pad.reshape(TJ, 128, D)
    TA_t = TA_pad.reshape(TJ, 128, C)

    cc = targets.sum(axis=0, dtype=np.float64) + 1.0  # [C]
    safe = cc > 1.5
    dcls = np.where(safe, 1.0 / np.maximum(cc - 1.0, 1.0) - 1.0 / cc, 0.0)
    invc = 1.0 / cc

    dmask_np = np.eye(128, dtype=f32).astype(BF)

    in_maps = []
    for k in range(CORES):
        rows = slice(k * R, (k + 1) * R)
        # permute j-tiles: this core's own 8 tiles go to positions 0..7
        own = list(range(8 * k, 8 * k + 8))
        perm = own + [t for t in range(TJ) if t not in own]
        fa_k = fa_t[perm]                      # [TJ, 128, D]
        TA_k = TA_t[perm]                      # [TJ, 128, C]

        fTg_np = np.ascontiguousarray(
            fa_k.transpose(2, 0, 1).reshape(D, NPAD)
        ).astype(BF)
        TA_kp = np.zeros((TJ, 128, CP), dtype=f32)
        TA_kp[:, :, :C] = TA_k
        TAg8_np = np.ascontiguousarray(TA_kp.transpose(1, 0, 2)).astype(F8NP)
        fTc_np = np.ascontiguousarray(features[rows].T).astype(BF)
        tTp_f32 = np.ascontiguousarray(targets[rows].T, dtype=f32)  # [C, R]
        W2_np = (dcls[:, None] * tTp_f32 + invc[:, None]).astype(BF)
        in_maps.append(
            {
                "fTc": fTc_np,
                "fTg": fTg_np,
                "dmask": dmask_np,
                "TAg8": TAg8_np,
                "W2": W2_np,
            }
        )
    return in_maps


def _run(centers1, features, targets, conf_mask, trace=False, trace_cores=None):
    f32 = np.float32
    features = np.ascontiguousarray(features, dtype=f32)
    centers1 = np.ascontiguousarray(centers1, dtype=f32).reshape(-1, D)
    targets = np.ascontiguousarray(targets, dtype=f32)
    conf = np.ascontiguousarray(conf_mask, dtype=np.float64)

    in_maps = _prep_inputs(centers1, features, targets)
    nc = _get_nc()
    kwargs = {}
    if trace:
        # NTFF profiling under axon: shim the (absent) antenv.axon_hooks
        # module and skip the artifact bucket upload.
        import types
        import concourse.bass_utils as bass_utils

        if "antenv.axon_hooks" not in sys.modules:
            mod = types.ModuleType("antenv.axon_hooks")
            mod._hook = None

            def set_axon_ntff_profile_hook(h):
                mod._hook = h

            def get_axon_ntff_profile_hook():
                return mod._hook

            mod.set_axon_ntff_profile_hook = set_axon_ntff_profile_hook
            mod.get_axon_ntff_profile_hook = get_axon_ntff_profile_hook
            sys.modules["antenv.axon_hooks"] = mod
            from trn_agent_boot.trn_boot import _ntff_profile_via_ctypes

            set_axon_ntff_profile_hook(
                _ntff_profile_via_ctypes("/opt/axon/libaxon_pjrt.so")
            )
        bass_utils.upload_artifacts = lambda tmpdir: "local://" + tmpdir
        kwargs = {"trace": True}
        if trace_cores is not None:
            kwargs["trace_cores"] = trace_cores
    res = run_bass_kernel_spmd(nc, in_maps, core_ids=list(range(CORES)), **kwargs)

    # host-side closed-form pieces (fp64, exact)
    S = np.concatenate([r["srow"][0] for r in res.results]).astype(np.float64)
    labs = targets.argmax(axis=1)
    cc = targets.sum(axis=0, dtype=np.float64) + 1.0
    gsum = targets.T.astype(np.float64) @ features.astype(np.float64)
    gsum += centers1.astype(np.float64)                      # [C, D]
    gath = gsum[labs]                                        # [B2, D]
    f64 = features.astype(np.float64)
    sm_raw = (f64 * gath).sum(axis=1) - (f64 * f64).sum(axis=1)
    SmT = (1.0 / TEMP) * sm_raw / (cc[labs] - 1.0)
    lnS = np.log(np.maximum(S, 1e-30)) + SH
    num = (conf * (lnS - SmT)).sum()
    den = conf.sum()
    loss = np.array(num / den, dtype=np.float32)
    return loss, res


def kernel(centers1, features, targets, cls_num_list, conf_mask):
    loss, _ = _run(centers1, features, targets, conf_mask)
    return loss
